# revision 61
# baseline (speedup 1.0000x reference)
"""Capsule routing layer (2 routing iterations) on 8 Trainium2 NeuronCores.

Reference computation:
    priors[b,o,i,h] = sum_d x[b,i,d] * W[o,d,h]          (never materialized)
    iter0: probs = softmax(0) = 1/O
           v0[b,o,h]  = (1/O) * sum_i priors
           out0       = squash(v0)
    logits[b,o,i]     = sum_h priors * out0
    iter1: probs      = softmax(logits, axis=o)
           v1[b,o,h]  = sum_i priors * probs
           return squash(v1)

Algebraic reduction used by this kernel (priors factors out of every use):
    xs[b,d]   = sum_i x[b,i,d]
    v0[b,o,h] = (1/O) sum_d xs[b,d] W[o,d,h]
    g0[b,o]   = sqrt(sn0)/(1+sn0),  sn0 = sum_h v0^2      (squash scale)
    w2[b,o,d] = sum_h W[o,d,h] (g0 * v0[b,o,h])           (g0 folded into v0)
    logits[b,o,i] = sum_d x[b,i,d] w2[b,o,d]
    p         = softmax_o(logits)
    xp[b,o,d] = sum_i p[b,o,i] x[b,i,d]
    v1[b,o,h] = sum_d xp[b,o,d] W[o,d,h]
    out       = squash(v1)

Sharding: data-parallel over batch B=64 across 8 cores, weights replicated.

Scheduling notes:
  - x arrives in two i-halves so the xs reduction starts sooner; xs is
    split across the DVE (b0-5) and the scalar engine's Identity+accum.
  - v0 and w2 run as o-pair matmuls (128-wide stationaries; for w2 the v0
    operand is zero-masked per o-parity) to halve stationary loads.
  - All logits matmuls are issued before all xp matmuls so the per-batch
    softmax (exp on ACT, esum/recip on DVE, probs mul alternating
    DVE/gpsimd) overlaps the PE stream; v1 runs in b-halves that fill PE
    gaps while later batches' xp still wait on probs.
  - The v1 squash works on the PE-transposed copy, where the h-sums are
    free-axis reduces; each 64-row output block flows transpose ->
    square -> sum -> g -> scale -> DMA independently.
  - Dummy activations prefetch the Sqrt/Exp table switches off the
    critical path.  All PSUM accumulation and the squash chains are fp32.
"""

import sys
from contextlib import ExitStack

for _p in ("/opt/trn_rl_repo", "/root/.axon_site/_ro/trn_rl_repo"):
    if _p not in sys.path:
        sys.path.append(_p)

import ml_dtypes
import numpy as np

import concourse.bacc as bacc
import concourse.tile as tile
from concourse import mybir
from concourse import bass_utils
from concourse.masks import make_identity
from concourse.bass import broadcast_tensor_aps

F32 = mybir.dt.float32
F16 = mybir.dt.float16
AF = mybir.ActivationFunctionType
NPF16 = np.float16

# Problem shape (hardcoded per spec)
B, I, DIN = 64, 512, 128
O, H = 32, 64
NCORES = 8
BL = B // NCORES          # 8 local batches per core
P = 128                   # SBUF partitions
IH = I // 2               # 256: i-half loaded per DMA
ITI = I // P              # 4 i-tiles of 128
BO = BL * O               # 256 (b,o) columns
NJ = O // 2               # 16 o-pairs
NT = BO // H              # 4 output row-blocks


def capsule_tile_kernel(tc, out_d, xtb_d, xnb_d, wfb_d, wtbP_d):
    with ExitStack() as ctx:
        _capsule_kernel(ctx, tc, out_d, xtb_d, xnb_d, wfb_d, wtbP_d)


def _capsule_kernel(ctx, tc, out_d, xtb_d, xnb_d, wfb_d, wtbP_d):
    nc = tc.nc

    consts = ctx.enter_context(tc.tile_pool(name="consts", bufs=1))
    data = ctx.enter_context(tc.tile_pool(name="data", bufs=1))
    small = ctx.enter_context(tc.tile_pool(name="small", bufs=1))
    pp = ctx.enter_context(tc.tile_pool(name="pp", bufs=1, space="PSUM"))
    plp = ctx.enter_context(tc.tile_pool(name="plp", bufs=3, space="PSUM"))
    efp = ctx.enter_context(tc.tile_pool(name="efp", bufs=3))

    # ---- constants (built during the DMA window) ----
    ident = consts.tile([H, H], F16)
    make_identity(nc, ident)
    # block-diagonal ones: partition-block reduce for the paired v0 layout
    bones = consts.tile([P, P], F16)
    nc.gpsimd.memset(bones, 1.0)
    nc.gpsimd.memset(bones[0:H, H:P], 0.0)
    nc.gpsimd.memset(bones[H:P, 0:H], 0.0)
    # zero-masked v0 staging (slot 0: even o, partitions 0-63; slot 1: odd)
    v0sz = data.tile([P, 2, P], F16)
    nc.gpsimd.memset(v0sz, 0.0)
    # ACT table prewarm scratch + sqrt bias constant
    dumin = consts.tile([1, 2], F32)
    nc.gpsimd.memset(dumin, 1.0)
    dumout = consts.tile([1, 2], F32)
    negone = consts.tile([P, 1], F32)
    nc.gpsimd.memset(negone, -1.0)

    # ---- loads (sync ring, priority order, single-free-dim views) ----
    xtb = data.tile([P, BL, I], F16)
    nc.sync.dma_start(out=xtb.rearrange("d b i -> d (b i)"),
                      in_=xtb_d.rearrange("d b i -> d (b i)"))
    wfb = consts.tile([P, O, H], F16)
    nc.sync.dma_start(out=wfb.rearrange("d o h -> d (o h)"),
                      in_=wfb_d.rearrange("d o h -> d (o h)"))
    # wtbP[(o%2)*64+h, o//2, d] = W[o,d,h]
    wtbP = consts.tile([P, NJ, DIN], F16)
    nc.sync.dma_start(out=wtbP.rearrange("p j d -> p (j d)"),
                      in_=wtbP_d.rearrange("p j d -> p (j d)"))
    # xnb[i, b*4+it, d] = x[b, it*128+i, d]
    xnb = data.tile([P, BL * ITI, DIN], F16)
    nc.sync.dma_start(out=xnb.rearrange("i e d -> i (e d)"),
                      in_=xnb_d.rearrange("i e d -> i (e d)"))

    # ---- xs[d, b] = sum_i x / 32: b0-5 on DVE, b6-7 on the scalar engine
    # (Identity activation with per-partition accumulator) ----
    xsf = small.tile([P, BL], F32)
    xsb = small.tile([P, BL], F16)
    xscr = data.tile([P, I], F16)
    nc.scalar.activation(dumout, dumin, AF.Sqrt)      # prefetch sqrt table
    nc.vector.reduce_sum(xsf[:, 0:4], xtb[:, 0:4], axis=mybir.AxisListType.X)
    nc.vector.tensor_scalar_mul(xsb[:, 0:4], xsf[:, 0:4], 1.0 / O)
    for b in range(4, BL):
        nc.scalar.activation(xscr, xtb[:, b], AF.Identity,
                             accum_out=xsf[:, b:b + 1])
    nc.vector.tensor_scalar_mul(xsb[:, 4:8], xsf[:, 4:8], 1.0 / O)

    # ---- head in two b-waves (w0 = b0-3, w1 = b4-7): xs -> v0 -> g0 -> w2
    # -> logits per wave, so wave-0's logits/softmax overlap wave-1's head.
    # v0 pair layout per wave: psv0[:, w] = [128=(o%2,h), 64=(j, b-in-wave)].
    # ACT order keeps both waves' Square/Sqrt before the Exp table load. ----
    wfb2 = wfb.rearrange("d o h -> d (o h)")
    xtv = xtb.rearrange("d b (t i) -> d b t i", i=P)
    psv0 = pp.tile([P, 2, H], F32, tag="v0")
    psg = pp.tile([P, 2, H], F32, tag="g")
    psw2 = pp.tile([P, BO], F32, tag="w2", bufs=2)
    w2v = psw2.rearrange("d (b o) -> d o b", o=O)
    sq0 = data.tile([P, 2, H], F16)
    rt0 = data.tile([P, 2, H], F32)
    dn0 = data.tile([P, 2, H], F32)
    rdn0 = data.tile([P, 2, H], F32)
    rscr = data.tile([P, 2, H], F32)
    g0p = data.tile([P, 2, H], F32)
    w2s = data.tile([P, BO], F16)
    psls = [None] * BL

    def v0_mms(w):
        for j in range(NJ):
            nc.tensor.matmul(psv0[:, w, j * 4:(j + 1) * 4],
                             wfb2[:, j * P:(j + 1) * P],
                             xsb[:, w * 4:(w + 1) * 4],
                             start=True, stop=True)

    def g0_act(w):
        nc.scalar.activation(sq0[:, w], psv0[:, w], AF.Square)
        nc.tensor.matmul(psg[:, w], bones, sq0[:, w], start=True, stop=True)
        nc.scalar.activation(rt0[:, w], psg[:, w], AF.Sqrt)

    def g0_dve(w):
        nc.vector.tensor_scalar_add(dn0[:, w], psg[:, w], 1.0)
        nc.vector.reciprocal_approx_accurate(rdn0[:, w], dn0[:, w],
                                             rscr[:, w])
        nc.vector.tensor_mul(g0p[:, w], rt0[:, w], rdn0[:, w])
        # masked scaled copies: v0sz[:, par, w] = g0*v0 for parity par, else 0
        nc.vector.tensor_mul(v0sz[0:H, 0, w * H:(w + 1) * H],
                             psv0[0:H, w, :], g0p[0:H, w, :])
        nc.vector.tensor_mul(v0sz[H:P, 1, w * H:(w + 1) * H],
                             psv0[H:P, w, :], g0p[H:P, w, :])

    def w2_mms(w):
        for j in range(NJ):
            sl = slice(w * H + j * 4, w * H + (j + 1) * 4)
            bsl = slice(w * 4, (w + 1) * 4)
            nc.tensor.matmul(w2v[:, 2 * j, bsl], wtbP[:, j, :],
                             v0sz[:, 0, sl], start=True, stop=True)
            nc.tensor.matmul(w2v[:, 2 * j + 1, bsl], wtbP[:, j, :],
                             v0sz[:, 1, sl], start=True, stop=True)

    def logits_mms(b):
        psl = plp.tile([P, ITI, O], F32, tag="psl", name=f"psl{b}")
        psls[b] = psl
        for it in range(ITI):
            nc.tensor.matmul(psl[:, it, :], xtv[:, b, it, :],
                             w2s[:, b * O:(b + 1) * O], start=True, stop=True)

    # softmax pieces (exp on ACT; esum/recip/probs on DVE)
    esum = small.tile([P, BL, ITI], F32)
    rs = small.tile([P, BL, ITI, 1], F32)
    probs = data.tile([P, BL, ITI, O], F16)
    psxp = pp.tile([P, BO], F32, tag="xp")
    xps = data.tile([P, BO], F16)
    psv1 = pp.tile([H, BO], F32, tag="v0")
    xpsv = xps.rearrange("d (b o) -> d o b", o=O)

    def softmax(b):
        ef = efp.tile([P, ITI, O], F32, tag="ef", name=f"ef{b}")
        nc.scalar.activation(ef, psls[b], AF.Exp)
        nc.vector.reduce_sum(esum[:, b], ef, axis=mybir.AxisListType.X)
        nc.vector.reciprocal(rs[:, b, :, 0], esum[:, b])
        ef_ap, rs_ap = broadcast_tensor_aps(ef[:], rs[:, b])
        nc.vector.tensor_mul(probs[:, b], ef_ap, rs_ap)

    def xp_mms(b):
        for it in range(ITI):
            nc.tensor.matmul(psxp[:, b * O:(b + 1) * O],
                             xnb[:, b * ITI + it, :], probs[:, b, it, :],
                             start=(it == 0), stop=(it == ITI - 1))

    v0_mms(0)
    g0_act(0)            # sq0/psg-mm/rt0 wave 0
    v0_mms(1)
    g0_act(1)            # both waves' Sqrt before the Exp table load
    nc.scalar.activation(dumout, dumin, AF.Exp)       # prefetch exp table
    g0_dve(0)
    w2_mms(0)
    nc.vector.tensor_copy(w2s[:, 0:P], psw2[:, 0:P])
    for b in range(4):
        logits_mms(b)
    g0_dve(1)
    w2_mms(1)
    for b in range(4):
        softmax(b)
    nc.vector.tensor_copy(w2s[:, P:BO], psw2[:, P:BO])
    for b in range(4, BL):
        logits_mms(b)
    for b in range(4):
        xp_mms(b)
    for b in range(4, BL):
        softmax(b)
        xp_mms(b)
    nc.scalar.activation(dumout, dumin, AF.Sqrt)      # re-prefetch sqrt

    for q in range(4):
        nc.vector.tensor_copy(xps[:, q * 64:(q + 1) * 64],
                              psxp[:, q * 64:(q + 1) * 64])
    for o in range(O):
        nc.tensor.matmul(psv1[:, o * BL:(o + 1) * BL], wfb[:, o, :],
                         xpsv[:, o, :], start=True, stop=True)

    # ---- transpose v1, then squash in [(o,b), h] layout; each 64-row
    # block flows transpose -> square -> sum -> g -> scale -> DMA ----
    v1s = data.tile([H, BO], F16)
    nc.vector.tensor_copy(v1s[:, 0:P], psv1[:, 0:P])
    nc.vector.tensor_copy(v1s[:, P:BO], psv1[:, P:BO])
    sqT = data.tile([H, NT, H], F16)
    snT = small.tile([H, NT], F32)
    rtT = small.tile([H, NT], F32)
    dnT = small.tile([H, NT], F32)
    rdnT = small.tile([H, NT], F32)
    gT = small.tile([H, NT, 1], F32)
    outT = data.tile([H, NT, H], F32)
    # ovv[o2, b, t, h] = out_d[b, t*8+o2, h]; flattened (o2,b,t,h) order
    # matches outT's (c=(o2,b), t, h) order element-for-element.
    ovv = out_d.rearrange("b (t o2) h -> o2 b t h", o2=BL)
    for t in range(NT):
        pso = pp.tile([H, H], F16, tag="w2", bufs=2)
        nc.tensor.transpose(pso, v1s[:, t * H:(t + 1) * H], ident)
        # sq = (v1/64)^2 straight from the transpose PSUM
        nc.scalar.activation(sqT[:, t, :], pso, AF.Square, scale=1.0 / 64)
        nc.vector.reduce_sum(snT[:, t:t + 1], sqT[:, t, :],
                             axis=mybir.AxisListType.X)
        # g1 = 64*sqrt(sn')/(1+4096*sn') = sqrt(sn1)/(1+sn1)  (sn'=sn1/4096)
        nc.scalar.activation(rtT[:, t:t + 1], snT[:, t:t + 1], AF.Sqrt)
        nc.vector.tensor_scalar(dnT[:, t:t + 1], snT[:, t:t + 1],
                                64.0, 1.0 / 64,
                                op0=mybir.AluOpType.mult,
                                op1=mybir.AluOpType.add)
        nc.vector.reciprocal(rdnT[:, t:t + 1], dnT[:, t:t + 1])
        nc.vector.tensor_mul(gT[:, t:t + 1, 0], rdnT[:, t:t + 1],
                             rtT[:, t:t + 1])
        v_ap, g_ap = broadcast_tensor_aps(pso[:], gT[:, t])
        nc.vector.tensor_mul(outT[:, t, :], v_ap, g_ap)
        eng = nc.sync if t % 2 == 0 else nc.scalar
        eng.dma_start(out=ovv[:, :, t, :], in_=outT[:, t, :])


def build_program():
    nc = bacc.Bacc("TRN2", debug=False, num_devices=NCORES)
    xtb_t = nc.dram_tensor("xtb", [P, BL, I], F16, kind="ExternalInput")
    xnb_t = nc.dram_tensor("xnb", [P, BL * ITI, DIN], F16,
                           kind="ExternalInput")
    wfb_t = nc.dram_tensor("wfb", [P, O, H], F16, kind="ExternalInput")
    wtbP_t = nc.dram_tensor("wtbP", [P, NJ, DIN], F16, kind="ExternalInput")
    out_t = nc.dram_tensor("out", [BL, O, H], F32, kind="ExternalOutput")
    with tile.TileContext(nc) as tc:
        capsule_tile_kernel(tc, out_t.ap(), xtb_t.ap(),
                            xnb_t.ap(), wfb_t.ap(), wtbP_t.ap())
    nc.compile()
    return nc


_program = None


def _get_program():
    global _program
    if _program is None:
        _program = build_program()
    return _program


def run_on_cores(x, route_weights, trace=False, **kwargs):
    """Run the SPMD kernel; returns (full_output, BassKernelResults)."""
    x = np.asarray(x, dtype=np.float32).astype(NPF16)
    w = np.asarray(route_weights, dtype=np.float32).astype(NPF16)
    nc = _get_program()
    wfb = np.ascontiguousarray(w.transpose(1, 0, 2))
    # wtbP[(o%2)*64+h, o//2, d] = W[o,d,h] (o-pair-packed W^T)
    wtbP = np.ascontiguousarray(
        w.reshape(NJ, 2, DIN, H).transpose(1, 3, 0, 2).reshape(P, NJ, DIN))
    in_maps = []
    for c in range(NCORES):
        xs = x[c * BL:(c + 1) * BL]
        xtb = np.ascontiguousarray(xs.transpose(2, 0, 1))
        # xnb[i, b*4+it, d] = x[b, it*128+i, d]
        xnb = np.ascontiguousarray(
            xs.reshape(BL * ITI, P, DIN).transpose(1, 0, 2))
        in_maps.append({"xtb": xtb, "xnb": xnb, "wfb": wfb, "wtbP": wtbP})
    res = bass_utils.run_bass_kernel_spmd(
        nc, in_maps, core_ids=list(range(NCORES)), trace=trace, **kwargs
    )
    out = np.concatenate([res.results[c]["out"] for c in range(NCORES)], axis=0)
    return out.astype(np.float32), res


def kernel(x, route_weights):
    out, _ = run_on_cores(x, route_weights)
    return out


# revision 62
# speedup vs baseline: 1.0585x; 1.0585x over previous
"""Capsule routing layer (2 routing iterations) on 8 Trainium2 NeuronCores.

Reference computation:
    priors[b,o,i,h] = sum_d x[b,i,d] * W[o,d,h]          (never materialized)
    iter0: probs = softmax(0) = 1/O
           v0[b,o,h]  = (1/O) * sum_i priors
           out0       = squash(v0)
    logits[b,o,i]     = sum_h priors * out0
    iter1: probs      = softmax(logits, axis=o)
           v1[b,o,h]  = sum_i priors * probs
           return squash(v1)

Algebraic reduction used by this kernel (priors factors out of every use):
    xs[b,d]   = sum_i x[b,i,d]
    v0[b,o,h] = (1/O) sum_d xs[b,d] W[o,d,h]
    g0[b,o]   = sqrt(sn0)/(1+sn0),  sn0 = sum_h v0^2      (squash scale)
    w2[b,o,d] = sum_h W[o,d,h] (g0 * v0[b,o,h])           (g0 folded into v0)
    logits[b,o,i] = sum_d x[b,i,d] w2[b,o,d]
    p         = softmax_o(logits)
    xp[b,o,d] = sum_i p[b,o,i] x[b,i,d]
    v1[b,o,h] = sum_d xp[b,o,d] W[o,d,h]
    out       = squash(v1)

Sharding: data-parallel over batch B=64 across 8 cores, weights replicated.

Scheduling notes:
  - x arrives in two i-halves so the xs reduction starts sooner; xs is
    split across the DVE (b0-5) and the scalar engine's Identity+accum.
  - v0 and w2 run as o-pair matmuls (128-wide stationaries; for w2 the v0
    operand is zero-masked per o-parity) to halve stationary loads.
  - All logits matmuls are issued before all xp matmuls so the per-batch
    softmax (exp on ACT, esum/recip on DVE, probs mul alternating
    DVE/gpsimd) overlaps the PE stream; v1 runs in b-halves that fill PE
    gaps while later batches' xp still wait on probs.
  - The v1 squash works on the PE-transposed copy, where the h-sums are
    free-axis reduces; each 64-row output block flows transpose ->
    square -> sum -> g -> scale -> DMA independently.
  - Dummy activations prefetch the Sqrt/Exp table switches off the
    critical path.  All PSUM accumulation and the squash chains are fp32.
"""

import sys
from contextlib import ExitStack

for _p in ("/opt/trn_rl_repo", "/root/.axon_site/_ro/trn_rl_repo"):
    if _p not in sys.path:
        sys.path.append(_p)

import ml_dtypes
import numpy as np

import concourse.bacc as bacc
import concourse.tile as tile
from concourse import mybir
from concourse import bass_utils
from concourse.masks import make_identity
from concourse.bass import broadcast_tensor_aps

F32 = mybir.dt.float32
F16 = mybir.dt.float16
AF = mybir.ActivationFunctionType
NPF16 = np.float16

# Problem shape (hardcoded per spec)
B, I, DIN = 64, 512, 128
O, H = 32, 64
NCORES = 8
BL = B // NCORES          # 8 local batches per core
P = 128                   # SBUF partitions
IH = I // 2               # 256: i-half loaded per DMA
ITI = I // P              # 4 i-tiles of 128
BO = BL * O               # 256 (b,o) columns
NJ = O // 2               # 16 o-pairs
NT = BO // H              # 4 output row-blocks


def capsule_tile_kernel(tc, out_d, xtb_d, xnb_d, wfb_d, wtbP_d):
    with ExitStack() as ctx:
        _capsule_kernel(ctx, tc, out_d, xtb_d, xnb_d, wfb_d, wtbP_d)


def _capsule_kernel(ctx, tc, out_d, xtb_d, xnb_d, wfb_d, wtbP_d):
    nc = tc.nc

    consts = ctx.enter_context(tc.tile_pool(name="consts", bufs=1))
    data = ctx.enter_context(tc.tile_pool(name="data", bufs=1))
    small = ctx.enter_context(tc.tile_pool(name="small", bufs=1))
    pp = ctx.enter_context(tc.tile_pool(name="pp", bufs=1, space="PSUM"))
    plp = ctx.enter_context(tc.tile_pool(name="plp", bufs=3, space="PSUM"))
    efp = ctx.enter_context(tc.tile_pool(name="efp", bufs=3))

    # ---- constants (built during the DMA window) ----
    ident = consts.tile([H, H], F16)
    make_identity(nc, ident)
    # block-diagonal ones: partition-block reduce for the paired v0 layout
    bones = consts.tile([P, P], F16)
    nc.gpsimd.memset(bones, 1.0)
    nc.gpsimd.memset(bones[0:H, H:P], 0.0)
    nc.gpsimd.memset(bones[H:P, 0:H], 0.0)
    # zero-masked v0 staging (slot 0: even o, partitions 0-63; slot 1: odd)
    v0sz = data.tile([P, 2, P], F16)
    nc.gpsimd.memset(v0sz, 0.0)
    # ACT table prewarm scratch + sqrt bias constant
    dumin = consts.tile([1, 2], F32)
    nc.gpsimd.memset(dumin, 1.0)
    dumout = consts.tile([1, 2], F32)
    negone = consts.tile([P, 1], F32)
    nc.gpsimd.memset(negone, -1.0)

    # ---- loads (sync ring, priority order, single-free-dim views) ----
    xtb = data.tile([P, BL, I], F16)
    nc.sync.dma_start(out=xtb.rearrange("d b i -> d (b i)"),
                      in_=xtb_d.rearrange("d b i -> d (b i)"))
    wfb = consts.tile([P, O, H], F16)
    nc.sync.dma_start(out=wfb.rearrange("d o h -> d (o h)"),
                      in_=wfb_d.rearrange("d o h -> d (o h)"))
    # wtbP[(o%2)*64+h, o//2, d] = W[o,d,h]
    wtbP = consts.tile([P, NJ, DIN], F16)
    nc.sync.dma_start(out=wtbP.rearrange("p j d -> p (j d)"),
                      in_=wtbP_d.rearrange("p j d -> p (j d)"))
    # xnb[i, b*4+it, d] = x[b, it*128+i, d]
    xnb = data.tile([P, BL * ITI, DIN], F16)
    nc.sync.dma_start(out=xnb.rearrange("i e d -> i (e d)"),
                      in_=xnb_d.rearrange("i e d -> i (e d)"))

    # ---- xs[d, b] = sum_i x / 32: b0-5 on DVE, b6-7 on the scalar engine
    # (Identity activation with per-partition accumulator) ----
    xsf = small.tile([P, BL], F32)
    xsb = small.tile([P, BL], F16)
    xscr = data.tile([P, I], F16)
    nc.scalar.activation(dumout, dumin, AF.Sqrt)      # prefetch sqrt table
    nc.vector.reduce_sum(xsf[:, 0:5], xtb[:, 0:5], axis=mybir.AxisListType.X)
    for b in range(5, BL):
        nc.scalar.activation(xscr, xtb[:, b], AF.Identity,
                             accum_out=xsf[:, b:b + 1])
    nc.vector.tensor_scalar_mul(xsb, xsf, 1.0 / O)

    # ---- v0 o-pairs: [128=(o%2,h), 128=(j,b)] = wfb-pair^T @ xs ----
    wfb2 = wfb.rearrange("d o h -> d (o h)")
    psv0 = pp.tile([P, P], F32, tag="v0")
    for j in range(NJ):
        nc.tensor.matmul(psv0[:, j * BL:(j + 1) * BL],
                         wfb2[:, j * P:(j + 1) * P], xsb,
                         start=True, stop=True)

    # ---- squash scale g0 (pair layout): psg = sn0 broadcast per o-parity
    # block; g0 = sqrt(sn0)/(1+sn0) ----
    sq0 = data.tile([P, P], F16)
    nc.scalar.activation(sq0, psv0, AF.Square)
    psg = pp.tile([P, P], F32, tag="g")
    nc.tensor.matmul(psg, bones, sq0, start=True, stop=True)
    rt0 = data.tile([P, P], F32)
    nc.scalar.activation(rt0, psg, AF.Sqrt)
    nc.scalar.activation(dumout, dumin, AF.Exp)       # prefetch exp table
    dn0 = data.tile([P, P], F32)
    nc.vector.tensor_scalar_add(dn0, psg, 1.0)
    rdn0 = data.tile([P, P], F32)
    rscr = data.tile([P, P], F32)
    nc.vector.reciprocal_approx_accurate(rdn0, dn0, rscr)
    g0p = data.tile([P, P], F32)
    nc.vector.tensor_mul(g0p, rt0, rdn0)

    # masked scaled copies: v0sz[:, par] holds g0*v0 for parity par, else 0
    nc.vector.tensor_mul(v0sz[0:H, 0, :], psv0[0:H, :], g0p[0:H, :])
    nc.vector.tensor_mul(v0sz[H:P, 1, :], psv0[H:P, :], g0p[H:P, :])

    # ---- w2[d,(b,o)] = wtbP_j^T @ masked v0 (contract (o%2,h)) ----
    psw2 = pp.tile([P, BO], F32, tag="w2", bufs=2)
    w2v = psw2.rearrange("d (b o) -> d o b", o=O)
    for j in range(NJ):
        sl = slice(j * BL, (j + 1) * BL)
        nc.tensor.matmul(w2v[:, 2 * j, :], wtbP[:, j, :], v0sz[:, 0, sl],
                         start=True, stop=True)
        nc.tensor.matmul(w2v[:, 2 * j + 1, :], wtbP[:, j, :], v0sz[:, 1, sl],
                         start=True, stop=True)
    w2s = data.tile([P, BO], F16)
    nc.vector.tensor_copy(w2s, psw2)

    # ---- logits for all b (PE) ----
    xtv = xtb.rearrange("d b (t i) -> d b t i", i=P)
    psls = []
    for b in range(BL):
        psl = plp.tile([P, ITI, O], F32, tag="psl")
        psls.append(psl)
        for it in range(ITI):
            nc.tensor.matmul(psl[:, it, :], xtv[:, b, it, :],
                             w2s[:, b * O:(b + 1) * O], start=True, stop=True)

    # ---- softmax over o (exp on ACT; esum/recip on DVE; probs mul
    # alternating DVE/gpsimd) interleaved with the xp matmuls; v1 runs in
    # b-halves that fill PE gaps while later batches wait on probs ----
    esum = small.tile([P, BL, ITI], F32)
    rs = small.tile([P, BL, ITI, 1], F32)
    probs = data.tile([P, BL, ITI, O], F16)
    psxp = pp.tile([P, BO], F32, tag="xp")
    xps = data.tile([P, BO], F16)
    psv1 = pp.tile([H, BO], F32, tag="v0")
    xpsv = xps.rearrange("d (b o) -> d o b", o=O)

    for b in range(BL):
        ef = efp.tile([P, ITI, O], F32, tag="ef", name=f"ef{b}")
        nc.scalar.activation(ef, psls[b], AF.Exp)
        nc.vector.reduce_sum(esum[:, b], ef, axis=mybir.AxisListType.X)
        nc.vector.reciprocal(rs[:, b, :, 0], esum[:, b])
        ef_ap, rs_ap = broadcast_tensor_aps(ef[:], rs[:, b])
        nc.vector.tensor_mul(probs[:, b], ef_ap, rs_ap)
    nc.scalar.activation(dumout, dumin, AF.Sqrt)      # re-prefetch sqrt

    for b in range(BL):
        for it in range(ITI):
            nc.tensor.matmul(psxp[:, b * O:(b + 1) * O],
                             xnb[:, b * ITI + it, :], probs[:, b, it, :],
                             start=(it == 0), stop=(it == ITI - 1))
    for q in range(4):
        nc.vector.tensor_copy(xps[:, q * 64:(q + 1) * 64],
                              psxp[:, q * 64:(q + 1) * 64])
    for o in range(O):
        nc.tensor.matmul(psv1[:, o * BL:(o + 1) * BL], wfb[:, o, :],
                         xpsv[:, o, :], start=True, stop=True)

    # ---- transpose v1, then squash in [(o,b), h] layout; each 64-row
    # block flows transpose -> square -> sum -> g -> scale -> DMA ----
    v1s = data.tile([H, BO], F16)
    nc.vector.tensor_copy(v1s[:, 0:P], psv1[:, 0:P])
    nc.vector.tensor_copy(v1s[:, P:BO], psv1[:, P:BO])
    sqT = data.tile([H, NT, H], F16)
    snT = small.tile([H, NT], F32)
    rtT = small.tile([H, NT], F32)
    dnT = small.tile([H, NT], F32)
    rdnT = small.tile([H, NT], F32)
    gT = small.tile([H, NT, 1], F32)
    outT = data.tile([H, NT, H], F32)
    # ovv[o2, b, t, h] = out_d[b, t*8+o2, h]; flattened (o2,b,t,h) order
    # matches outT's (c=(o2,b), t, h) order element-for-element.
    ovv = out_d.rearrange("b (t o2) h -> o2 b t h", o2=BL)
    for t in range(NT):
        pso = pp.tile([H, H], F16, tag="w2", bufs=2)
        nc.tensor.transpose(pso, v1s[:, t * H:(t + 1) * H], ident)
        # sq = (v1/64)^2 straight from the transpose PSUM
        nc.scalar.activation(sqT[:, t, :], pso, AF.Square, scale=1.0 / 64)
        nc.vector.reduce_sum(snT[:, t:t + 1], sqT[:, t, :],
                             axis=mybir.AxisListType.X)
        # g1 = 64*sqrt(sn')/(1+4096*sn') = sqrt(sn1)/(1+sn1)  (sn'=sn1/4096)
        nc.scalar.activation(rtT[:, t:t + 1], snT[:, t:t + 1], AF.Sqrt)
        nc.vector.tensor_scalar(dnT[:, t:t + 1], snT[:, t:t + 1],
                                64.0, 1.0 / 64,
                                op0=mybir.AluOpType.mult,
                                op1=mybir.AluOpType.add)
        nc.vector.reciprocal(rdnT[:, t:t + 1], dnT[:, t:t + 1])
        nc.vector.tensor_mul(gT[:, t:t + 1, 0], rdnT[:, t:t + 1],
                             rtT[:, t:t + 1])
        v_ap, g_ap = broadcast_tensor_aps(pso[:], gT[:, t])
        nc.vector.tensor_mul(outT[:, t, :], v_ap, g_ap)
        eng = nc.sync if t % 2 == 0 else nc.scalar
        eng.dma_start(out=ovv[:, :, t, :], in_=outT[:, t, :])


def build_program():
    nc = bacc.Bacc("TRN2", debug=False, num_devices=NCORES)
    xtb_t = nc.dram_tensor("xtb", [P, BL, I], F16, kind="ExternalInput")
    xnb_t = nc.dram_tensor("xnb", [P, BL * ITI, DIN], F16,
                           kind="ExternalInput")
    wfb_t = nc.dram_tensor("wfb", [P, O, H], F16, kind="ExternalInput")
    wtbP_t = nc.dram_tensor("wtbP", [P, NJ, DIN], F16, kind="ExternalInput")
    out_t = nc.dram_tensor("out", [BL, O, H], F32, kind="ExternalOutput")
    with tile.TileContext(nc) as tc:
        capsule_tile_kernel(tc, out_t.ap(), xtb_t.ap(),
                            xnb_t.ap(), wfb_t.ap(), wtbP_t.ap())
    nc.compile()
    return nc


_program = None


def _get_program():
    global _program
    if _program is None:
        _program = build_program()
    return _program


def run_on_cores(x, route_weights, trace=False, **kwargs):
    """Run the SPMD kernel; returns (full_output, BassKernelResults)."""
    x = np.asarray(x, dtype=np.float32).astype(NPF16)
    w = np.asarray(route_weights, dtype=np.float32).astype(NPF16)
    nc = _get_program()
    wfb = np.ascontiguousarray(w.transpose(1, 0, 2))
    # wtbP[(o%2)*64+h, o//2, d] = W[o,d,h] (o-pair-packed W^T)
    wtbP = np.ascontiguousarray(
        w.reshape(NJ, 2, DIN, H).transpose(1, 3, 0, 2).reshape(P, NJ, DIN))
    in_maps = []
    for c in range(NCORES):
        xs = x[c * BL:(c + 1) * BL]
        xtb = np.ascontiguousarray(xs.transpose(2, 0, 1))
        # xnb[i, b*4+it, d] = x[b, it*128+i, d]
        xnb = np.ascontiguousarray(
            xs.reshape(BL * ITI, P, DIN).transpose(1, 0, 2))
        in_maps.append({"xtb": xtb, "xnb": xnb, "wfb": wfb, "wtbP": wtbP})
    res = bass_utils.run_bass_kernel_spmd(
        nc, in_maps, core_ids=list(range(NCORES)), trace=trace, **kwargs
    )
    out = np.concatenate([res.results[c]["out"] for c in range(NCORES)], axis=0)
    return out.astype(np.float32), res


def kernel(x, route_weights):
    out, _ = run_on_cores(x, route_weights)
    return out


# revision 63
# speedup vs baseline: 1.0737x; 1.0143x over previous
"""Capsule routing layer (2 routing iterations) on 8 Trainium2 NeuronCores.

Reference computation:
    priors[b,o,i,h] = sum_d x[b,i,d] * W[o,d,h]          (never materialized)
    iter0: probs = softmax(0) = 1/O
           v0[b,o,h]  = (1/O) * sum_i priors
           out0       = squash(v0)
    logits[b,o,i]     = sum_h priors * out0
    iter1: probs      = softmax(logits, axis=o)
           v1[b,o,h]  = sum_i priors * probs
           return squash(v1)

Algebraic reduction used by this kernel (priors factors out of every use):
    xs[b,d]   = sum_i x[b,i,d]
    v0[b,o,h] = (1/O) sum_d xs[b,d] W[o,d,h]
    g0[b,o]   = sqrt(sn0)/(1+sn0),  sn0 = sum_h v0^2      (squash scale)
    w2[b,o,d] = sum_h W[o,d,h] (g0 * v0[b,o,h])           (g0 folded into v0)
    logits[b,o,i] = sum_d x[b,i,d] w2[b,o,d]
    p         = softmax_o(logits)
    xp[b,o,d] = sum_i p[b,o,i] x[b,i,d]
    v1[b,o,h] = sum_d xp[b,o,d] W[o,d,h]
    out       = squash(v1)

Sharding: data-parallel over batch B=64 across 8 cores, weights replicated.

Scheduling notes:
  - x arrives in two i-halves so the xs reduction starts sooner; xs is
    split across the DVE (b0-5) and the scalar engine's Identity+accum.
  - v0 and w2 run as o-pair matmuls (128-wide stationaries; for w2 the v0
    operand is zero-masked per o-parity) to halve stationary loads.
  - All logits matmuls are issued before all xp matmuls so the per-batch
    softmax (exp on ACT, esum/recip on DVE, probs mul alternating
    DVE/gpsimd) overlaps the PE stream; v1 runs in b-halves that fill PE
    gaps while later batches' xp still wait on probs.
  - The v1 squash works on the PE-transposed copy, where the h-sums are
    free-axis reduces; each 64-row output block flows transpose ->
    square -> sum -> g -> scale -> DMA independently.
  - Dummy activations prefetch the Sqrt/Exp table switches off the
    critical path.  All PSUM accumulation and the squash chains are fp32.
"""

import sys
from contextlib import ExitStack

for _p in ("/opt/trn_rl_repo", "/root/.axon_site/_ro/trn_rl_repo"):
    if _p not in sys.path:
        sys.path.append(_p)

import ml_dtypes
import numpy as np

import concourse.bacc as bacc
import concourse.tile as tile
from concourse import mybir
from concourse import bass_utils
from concourse.masks import make_identity
from concourse.bass import broadcast_tensor_aps

F32 = mybir.dt.float32
F16 = mybir.dt.float16
AF = mybir.ActivationFunctionType
NPF16 = np.float16

# Problem shape (hardcoded per spec)
B, I, DIN = 64, 512, 128
O, H = 32, 64
NCORES = 8
BL = B // NCORES          # 8 local batches per core
P = 128                   # SBUF partitions
IH = I // 2               # 256: i-half loaded per DMA
ITI = I // P              # 4 i-tiles of 128
BO = BL * O               # 256 (b,o) columns
NJ = O // 2               # 16 o-pairs
NT = BO // H              # 4 output row-blocks


def capsule_tile_kernel(tc, out_d, xtb_d, xnb_d, wfb_d, wtbP_d):
    with ExitStack() as ctx:
        _capsule_kernel(ctx, tc, out_d, xtb_d, xnb_d, wfb_d, wtbP_d)


def _capsule_kernel(ctx, tc, out_d, xtb_d, xnb_d, wfb_d, wtbP_d):
    nc = tc.nc

    consts = ctx.enter_context(tc.tile_pool(name="consts", bufs=1))
    data = ctx.enter_context(tc.tile_pool(name="data", bufs=1))
    small = ctx.enter_context(tc.tile_pool(name="small", bufs=1))
    pp = ctx.enter_context(tc.tile_pool(name="pp", bufs=1, space="PSUM"))
    plp = ctx.enter_context(tc.tile_pool(name="plp", bufs=3, space="PSUM"))
    efp = ctx.enter_context(tc.tile_pool(name="efp", bufs=4))

    # ---- constants (built during the DMA window) ----
    ident = consts.tile([H, H], F16)
    make_identity(nc, ident)
    # block-diagonal ones: partition-block reduce for the paired v0 layout
    bones = consts.tile([P, P], F16)
    nc.gpsimd.memset(bones, 1.0)
    nc.gpsimd.memset(bones[0:H, H:P], 0.0)
    nc.gpsimd.memset(bones[H:P, 0:H], 0.0)
    # zero-masked v0 staging (slot 0: even o, partitions 0-63; slot 1: odd)
    v0sz = data.tile([P, 2, P], F16)
    nc.gpsimd.memset(v0sz, 0.0)
    # ACT table prewarm scratch + sqrt bias constant
    dumin = consts.tile([1, 2], F32)
    nc.gpsimd.memset(dumin, 1.0)
    dumout = consts.tile([1, 2], F32)
    negone = consts.tile([P, 1], F32)
    nc.gpsimd.memset(negone, -1.0)

    # ---- loads (sync ring, priority order, single-free-dim views) ----
    xtb = data.tile([P, BL, I], F16)
    nc.sync.dma_start(out=xtb.rearrange("d b i -> d (b i)"),
                      in_=xtb_d.rearrange("d b i -> d (b i)"))
    wfb = consts.tile([P, O, H], F16)
    nc.sync.dma_start(out=wfb.rearrange("d o h -> d (o h)"),
                      in_=wfb_d.rearrange("d o h -> d (o h)"))
    # wtbP[(o%2)*64+h, o//2, d] = W[o,d,h]
    wtbP = consts.tile([P, NJ, DIN], F16)
    nc.sync.dma_start(out=wtbP.rearrange("p j d -> p (j d)"),
                      in_=wtbP_d.rearrange("p j d -> p (j d)"))
    # xnb[i, b*4+it, d] = x[b, it*128+i, d]
    xnb = data.tile([P, BL * ITI, DIN], F16)
    nc.sync.dma_start(out=xnb.rearrange("i e d -> i (e d)"),
                      in_=xnb_d.rearrange("i e d -> i (e d)"))

    # ---- xs[d, b] = sum_i x / 32: b0-5 on DVE, b6-7 on the scalar engine
    # (Identity activation with per-partition accumulator) ----
    xsf = small.tile([P, BL], F32)
    xsb = small.tile([P, BL], F16)
    xscr = data.tile([P, I], F16)
    nc.scalar.activation(dumout, dumin, AF.Sqrt)      # prefetch sqrt table
    nc.vector.reduce_sum(xsf[:, 0:5], xtb[:, 0:5], axis=mybir.AxisListType.X)
    for b in range(5, BL):
        nc.scalar.activation(xscr, xtb[:, b], AF.Identity,
                             accum_out=xsf[:, b:b + 1])
    nc.vector.tensor_scalar_mul(xsb, xsf, 1.0 / O)

    # ---- v0 o-pairs: [128=(o%2,h), 128=(j,b)] = wfb-pair^T @ xs ----
    wfb2 = wfb.rearrange("d o h -> d (o h)")
    psv0 = pp.tile([P, P], F32, tag="v0")
    for j in range(NJ):
        nc.tensor.matmul(psv0[:, j * BL:(j + 1) * BL],
                         wfb2[:, j * P:(j + 1) * P], xsb,
                         start=True, stop=True)

    # ---- squash scale g0 (pair layout): psg = sn0 broadcast per o-parity
    # block; g0 = sqrt(sn0)/(1+sn0) ----
    sq0 = data.tile([P, P], F16)
    nc.scalar.activation(sq0, psv0, AF.Square)
    psg = pp.tile([P, P], F32, tag="g")
    nc.tensor.matmul(psg, bones, sq0, start=True, stop=True)
    rt0 = data.tile([P, P], F32)
    nc.scalar.activation(rt0, psg, AF.Sqrt)
    nc.scalar.activation(dumout, dumin, AF.Exp)       # prefetch exp table
    dn0 = data.tile([P, P], F32)
    nc.vector.tensor_scalar_add(dn0, psg, 1.0)
    rdn0 = data.tile([P, P], F32)
    rscr = data.tile([P, P], F32)
    nc.vector.reciprocal_approx_accurate(rdn0, dn0, rscr)
    g0p = data.tile([P, P], F32)
    nc.vector.tensor_mul(g0p, rt0, rdn0)

    # masked scaled copies: v0sz[:, par] holds g0*v0 for parity par, else 0
    nc.vector.tensor_mul(v0sz[0:H, 0, :], psv0[0:H, :], g0p[0:H, :])
    nc.vector.tensor_mul(v0sz[H:P, 1, :], psv0[H:P, :], g0p[H:P, :])

    # ---- w2[d,(b,o)] = wtbP_j^T @ masked v0 (contract (o%2,h)) ----
    psw2 = pp.tile([P, BO], F32, tag="w2", bufs=2)
    w2v = psw2.rearrange("d (b o) -> d o b", o=O)
    for j in range(NJ):
        sl = slice(j * BL, (j + 1) * BL)
        nc.tensor.matmul(w2v[:, 2 * j, :], wtbP[:, j, :], v0sz[:, 0, sl],
                         start=True, stop=True)
        nc.tensor.matmul(w2v[:, 2 * j + 1, :], wtbP[:, j, :], v0sz[:, 1, sl],
                         start=True, stop=True)
    w2s = data.tile([P, BO], F16)
    nc.vector.tensor_copy(w2s, psw2)

    # ---- logits for all b (PE) ----
    xtv = xtb.rearrange("d b (t i) -> d b t i", i=P)
    psls = []
    for b in range(BL):
        if b % 4 == 3:                    # 4th slot: reuse the dead psg bank
            psl = pp.tile([P, ITI, O], F32, tag="g", name=f"psl{b}")
        else:
            psl = plp.tile([P, ITI, O], F32, tag="psl", name=f"psl{b}")
        psls.append(psl)
        for it in range(ITI):
            nc.tensor.matmul(psl[:, it, :], xtv[:, b, it, :],
                             w2s[:, b * O:(b + 1) * O], start=True, stop=True)

    # ---- softmax over o (exp on ACT; esum/recip on DVE; probs mul
    # alternating DVE/gpsimd) interleaved with the xp matmuls; v1 runs in
    # b-halves that fill PE gaps while later batches wait on probs ----
    esum = small.tile([P, BL, ITI], F32)
    rs = small.tile([P, BL, ITI, 1], F32)
    probs = data.tile([P, BL, ITI, O], F16)
    psxp = pp.tile([P, BO], F32, tag="xp")
    xps = data.tile([P, BO], F16)
    psv1 = pp.tile([H, BO], F32, tag="v0")
    xpsv = xps.rearrange("d (b o) -> d o b", o=O)

    for b in range(BL):
        ef = efp.tile([P, ITI, O], F32, tag="ef", name=f"ef{b}")
        nc.scalar.activation(ef, psls[b], AF.Exp)
        nc.vector.reduce_sum(esum[:, b], ef, axis=mybir.AxisListType.X)
        nc.vector.reciprocal(rs[:, b, :, 0], esum[:, b])
        ef_ap, rs_ap = broadcast_tensor_aps(ef[:], rs[:, b])
        nc.vector.tensor_mul(probs[:, b], ef_ap, rs_ap)
    nc.scalar.activation(dumout, dumin, AF.Sqrt)      # re-prefetch sqrt

    for b in range(BL):
        for it in range(ITI):
            nc.tensor.matmul(psxp[:, b * O:(b + 1) * O],
                             xnb[:, b * ITI + it, :], probs[:, b, it, :],
                             start=(it == 0), stop=(it == ITI - 1))
    for q in range(4):
        nc.vector.tensor_copy(xps[:, q * 64:(q + 1) * 64],
                              psxp[:, q * 64:(q + 1) * 64])
    for o in range(O):
        nc.tensor.matmul(psv1[:, o * BL:(o + 1) * BL], wfb[:, o, :],
                         xpsv[:, o, :], start=True, stop=True)

    # ---- transpose v1, then squash in [(o,b), h] layout; each 64-row
    # block flows transpose -> square -> sum -> g -> scale -> DMA ----
    v1s = data.tile([H, BO], F16)
    nc.vector.tensor_copy(v1s[:, 0:P], psv1[:, 0:P])
    nc.vector.tensor_copy(v1s[:, P:BO], psv1[:, P:BO])
    sqT = data.tile([H, NT, H], F16)
    snT = small.tile([H, NT], F32)
    rtT = small.tile([H, NT], F32)
    dnT = small.tile([H, NT], F32)
    rdnT = small.tile([H, NT], F32)
    gT = small.tile([H, NT, 1], F32)
    outT = data.tile([H, NT, H], F32)
    # ovv[o2, b, t, h] = out_d[b, t*8+o2, h]; flattened (o2,b,t,h) order
    # matches outT's (c=(o2,b), t, h) order element-for-element.
    ovv = out_d.rearrange("b (t o2) h -> o2 b t h", o2=BL)
    for t in range(NT):
        pso = pp.tile([H, H], F16, tag="w2", bufs=2)
        nc.tensor.transpose(pso, v1s[:, t * H:(t + 1) * H], ident)
        # sq = (v1/64)^2 straight from the transpose PSUM
        nc.scalar.activation(sqT[:, t, :], pso, AF.Square, scale=1.0 / 64)
        nc.vector.reduce_sum(snT[:, t:t + 1], sqT[:, t, :],
                             axis=mybir.AxisListType.X)
        # g1 = 64*sqrt(sn')/(1+4096*sn') = sqrt(sn1)/(1+sn1)  (sn'=sn1/4096)
        nc.scalar.activation(rtT[:, t:t + 1], snT[:, t:t + 1], AF.Sqrt)
        nc.vector.tensor_scalar(dnT[:, t:t + 1], snT[:, t:t + 1],
                                64.0, 1.0 / 64,
                                op0=mybir.AluOpType.mult,
                                op1=mybir.AluOpType.add)
        nc.vector.reciprocal(rdnT[:, t:t + 1], dnT[:, t:t + 1])
        nc.vector.tensor_mul(gT[:, t:t + 1, 0], rdnT[:, t:t + 1],
                             rtT[:, t:t + 1])
        v_ap, g_ap = broadcast_tensor_aps(pso[:], gT[:, t])
        nc.vector.tensor_mul(outT[:, t, :], v_ap, g_ap)
        eng = nc.sync if t % 2 == 0 else nc.scalar
        eng.dma_start(out=ovv[:, :, t, :], in_=outT[:, t, :])


def build_program():
    nc = bacc.Bacc("TRN2", debug=False, num_devices=NCORES)
    xtb_t = nc.dram_tensor("xtb", [P, BL, I], F16, kind="ExternalInput")
    xnb_t = nc.dram_tensor("xnb", [P, BL * ITI, DIN], F16,
                           kind="ExternalInput")
    wfb_t = nc.dram_tensor("wfb", [P, O, H], F16, kind="ExternalInput")
    wtbP_t = nc.dram_tensor("wtbP", [P, NJ, DIN], F16, kind="ExternalInput")
    out_t = nc.dram_tensor("out", [BL, O, H], F32, kind="ExternalOutput")
    with tile.TileContext(nc) as tc:
        capsule_tile_kernel(tc, out_t.ap(), xtb_t.ap(),
                            xnb_t.ap(), wfb_t.ap(), wtbP_t.ap())
    nc.compile()
    return nc


_program = None


def _get_program():
    global _program
    if _program is None:
        _program = build_program()
    return _program


def run_on_cores(x, route_weights, trace=False, **kwargs):
    """Run the SPMD kernel; returns (full_output, BassKernelResults)."""
    x = np.asarray(x, dtype=np.float32).astype(NPF16)
    w = np.asarray(route_weights, dtype=np.float32).astype(NPF16)
    nc = _get_program()
    wfb = np.ascontiguousarray(w.transpose(1, 0, 2))
    # wtbP[(o%2)*64+h, o//2, d] = W[o,d,h] (o-pair-packed W^T)
    wtbP = np.ascontiguousarray(
        w.reshape(NJ, 2, DIN, H).transpose(1, 3, 0, 2).reshape(P, NJ, DIN))
    in_maps = []
    for c in range(NCORES):
        xs = x[c * BL:(c + 1) * BL]
        xtb = np.ascontiguousarray(xs.transpose(2, 0, 1))
        # xnb[i, b*4+it, d] = x[b, it*128+i, d]
        xnb = np.ascontiguousarray(
            xs.reshape(BL * ITI, P, DIN).transpose(1, 0, 2))
        in_maps.append({"xtb": xtb, "xnb": xnb, "wfb": wfb, "wtbP": wtbP})
    res = bass_utils.run_bass_kernel_spmd(
        nc, in_maps, core_ids=list(range(NCORES)), trace=trace, **kwargs
    )
    out = np.concatenate([res.results[c]["out"] for c in range(NCORES)], axis=0)
    return out.astype(np.float32), res


def kernel(x, route_weights):
    out, _ = run_on_cores(x, route_weights)
    return out
